# revision 19
# baseline (speedup 1.0000x reference)
"""Trainium2 Bass kernel for nn_CACLayer (retrieval + softmax readout + CE).

Computation (see reference):
  att = (q @ db.T) / sqrt(D); w = softmax(att, -1); z = w @ db
  logits = z @ fc_w.T + fc_b; nll = -log_softmax(logits)[targets]; out = mean(nll)

Strategy: data-parallel over batch B=2048 across 8 cores (256 queries each).
All heavy matmuls run in fp8e4m3 with DoubleRow perf mode (k=256 contraction,
~1.7x the bf16 column rate).  Softmax is computed un-normalized with the exp
biased by -ln(32) so the fp8 weights stay inside e4m3 range (softmax is
invariant to that scaling); the softmax sums accumulate on the DVE and Pool
engines so the PE only runs the att / z / classifier matmuls.  z is
normalized and rescaled by SZ=128 into fp8; fc_w is pre-scaled by SF=32 on
the host; the classifier exp folds the 1/(SZ*SF) back in via its scale.
"""

import os
import sys

for _p in ("/opt/trn_rl_repo", "/root/.axon_site/_ro/trn_rl_repo"):
    if os.path.isdir(_p) and _p not in sys.path:
        sys.path.insert(0, _p)

import math
import numpy as np
import ml_dtypes

import concourse.bass as bass
from concourse import bacc, mybir, tile
from concourse.bass_utils import run_bass_kernel_spmd
from concourse.masks import make_identity

BF16 = mybir.dt.bfloat16
F32 = mybir.dt.float32
FP8 = mybir.dt.float8e4
AF = mybir.ActivationFunctionType
ALU = mybir.AluOpType
AX = mybir.AxisListType
DR = mybir.MatmulPerfMode.DoubleRow

D = 512          # embed dim
N_DB = 32768     # database rows
B = 2048         # batch
C = 10000        # classes
N_CORES = 8
NQ = B // N_CORES          # queries per core (256)
QT = NQ // 128             # q tiles per core (2)
DS = D // 128              # d slices (4)
NST = N_DB // 512          # supertiles of 4 n-tiles (64)
TAU = float(D) ** -0.5
WBIAS = -math.log(32.0)    # exp output scale: keeps w = exp(att)/32 < 240
SZ = 128.0                 # z quantization scale
SF = 32.0                  # fc_w quantization scale
CW = 500                   # classifier chunk width (20 chunks x 500 = C)
NCH = C // CW              # 20 chunks
WAVES = NCH // 2           # CE waves of 2 chunks

_CACHE = {}


def build_nc(nst=NST, nch=NCH, nq=NQ, repeat=1, stream_bufs=4, wexp_bufs=3,
             no_ce=False, no_s=False, no_dma=False, with_bias=False):
    """Build the Bass module.  Parameterized so a scaled-down version can be
    simulated; hardware uses the defaults."""
    qt = nq // 128
    n_classes = nch * CW
    waves = nch // 2

    nc = bacc.Bacc("TRN2", target_bir_lowering=False, debug=False)

    qT_d = nc.dram_tensor("qT", [128, DS, nq], FP8, kind="ExternalInput")
    dbT_d = nc.dram_tensor("dbT", [nst, 128, 4, DS, 128], FP8, kind="ExternalInput")
    db_d = nc.dram_tensor("db", [nst, 128, 4, D], FP8, kind="ExternalInput")
    fcw_d = nc.dram_tensor("fcw", [128, DS, n_classes], FP8, kind="ExternalInput")
    sep_d = nc.dram_tensor("sep", [128, qt, waves], F32, kind="ExternalOutput")
    zq_d = nc.dram_tensor("zq", [128, DS, nq], FP8, kind="ExternalOutput")
    if with_bias:
        fcb_d = nc.dram_tensor("fcb", [1, n_classes], BF16, kind="ExternalInput")

    with tile.TileContext(nc) as tc:
        with (
            tc.tile_pool(name="const", bufs=1) as cpool,
            tc.tile_pool(name="stream", bufs=stream_bufs) as spool,
            tc.tile_pool(name="wexp", bufs=wexp_bufs) as wpool,
            tc.tile_pool(name="psA", bufs=2, space="PSUM") as psA,
            tc.tile_pool(name="psAcc", bufs=1, space="PSUM") as psAcc,
        ):
            # ---- resident tensors ----
            qT_sb = cpool.tile([128, DS, nq], FP8)
            nc.sync.dma_start(qT_sb[:], qT_d[:])
            fcw_sb = cpool.tile([128, DS, n_classes], FP8)
            for ds in range(DS):
                nc.sync.dma_start(fcw_sb[:, ds], fcw_d[:, ds])
            if with_bias:
                fcb_sb = cpool.tile([1, n_classes], BF16)
                nc.sync.dma_start(fcb_sb[:], fcb_d[:])
                ones1_bf = cpool.tile([1, 128], BF16)
                nc.vector.memset(ones1_bf[:], 1.0)

            onesc_f32 = cpool.tile([128, 1], F32)
            nc.vector.memset(onesc_f32[:], 1.0)
            ones1_f32 = cpool.tile([1, 128], F32)
            nc.vector.memset(ones1_f32[:], 1.0)
            wbias_sb = cpool.tile([128, 1], F32)
            nc.vector.memset(wbias_sb[:], WBIAS)

            # ---- phase A: att -> exp -> z accumulation over db ----
            # zT_ps [128(d_in), DS, nq] : 2 psum banks
            zT_ps = psAcc.tile([128, DS, nq], F32)

            import contextlib
            rep_cm = tc.For_i(0, repeat, 1) if repeat > 1 else contextlib.nullcontext()
            with rep_cm:
                # 2-stage software pipeline over supertiles: while the ACT
                # engine computes exp(st), the PE runs att matmuls of st+1,
                # so the PE never stalls on the exp dependency.
                db_tiles = {}

                def _load(st):
                    if no_dma and st > 0:
                        db_tiles[st] = db_tiles[0]
                        return
                    dbT_sb = spool.tile([128, 4, DS, 128], FP8, tag="dbT")
                    db_sb = spool.tile([128, 4, D], FP8, tag="db")
                    nc.sync.dma_start(dbT_sb[:], dbT_d[st])
                    nc.sync.dma_start(db_sb[:], db_d[st])
                    db_tiles[st] = (dbT_sb, db_sb)

                att_tiles = {}

                def _att(st):
                    dbT_sb = db_tiles[st][0]
                    att_ps = psA.tile([128, 4, nq], F32, tag="att")
                    # att_ps spans 2 psum banks (j 0,1 / j 2,3): one
                    # accumulation group per bank
                    for j in range(4):
                        for dp in range(2):
                            nc.tensor.matmul(
                                att_ps[:, j, :],
                                lhsT=dbT_sb[:, j, 2 * dp:2 * dp + 2, :],
                                rhs=qT_sb[:, 2 * dp:2 * dp + 2, :],
                                start=(dp == 0 and j % 2 == 0),
                                stop=(dp == 1 and j % 2 == 1),
                                perf_mode=DR,
                            )
                    att_tiles[st] = att_ps

                # softmax-sum accumulators (DVE + Pool), reset via copy at st=0
                s_accv = cpool.tile([128, nq], F32)
                s_accp = cpool.tile([128, nq], F32)

                _load(0)
                _load(1)
                _att(0)
                for st in range(nst):
                    if st + 2 < nst:
                        _load(st + 2)
                    w_sb = wpool.tile([128, 4, nq], FP8, tag="w")
                    att_ps = att_tiles.pop(st)
                    # two activations (one per psum bank) so z(st) jp0 only
                    # waits on the first half of the exp
                    for h in range(2):
                        nc.scalar.activation(
                            w_sb[:, 2 * h:2 * h + 2, :],
                            att_ps[:, 2 * h:2 * h + 2, :],
                            AF.Exp, scale=TAU, bias=wbias_sb[:],
                        )
                    if st + 1 < nst:
                        _att(st + 1)
                    db_sb = db_tiles[st][1]
                    if not no_dma:
                        del db_tiles[st]
                    for jp in range(2):
                        for ds in range(DS):
                            nc.tensor.matmul(
                                zT_ps[:, ds, :],
                                lhsT=db_sb[:, 2 * jp:2 * jp + 2,
                                           ds * 128:(ds + 1) * 128],
                                rhs=w_sb[:, 2 * jp:2 * jp + 2, :],
                                start=(st == 0 and jp == 0 and ds in (0, 2)),
                                stop=(st == nst - 1 and jp == 1
                                      and ds in (1, 3)),
                                perf_mode=DR,
                            )
                    if not no_s:
                        if st == 0:
                            nc.vector.tensor_copy(s_accv[:], w_sb[:, 0, :])
                            nc.vector.tensor_tensor(
                                s_accv[:], s_accv[:], w_sb[:, 1, :], ALU.add)
                            nc.gpsimd.tensor_copy(s_accp[:], w_sb[:, 2, :])
                            nc.gpsimd.tensor_tensor(
                                s_accp[:], s_accp[:], w_sb[:, 3, :], ALU.add)
                        else:
                            for j, eng, acc in ((0, nc.vector, s_accv),
                                                (1, nc.vector, s_accv),
                                                (2, nc.gpsimd, s_accp),
                                                (3, nc.gpsimd, s_accp)):
                                eng.tensor_tensor(
                                    acc[:], acc[:], w_sb[:, j, :], ALU.add)

                # ---- softmax normalization of z (scaled by SZ into fp8) ----
                s_sum = cpool.tile([128, nq], F32)
                if no_s:
                    nc.vector.memset(s_sum[:], 1.0)
                else:
                    nc.vector.tensor_tensor(
                        s_sum[:], s_accv[:], s_accp[:], ALU.add)
                s_ps = psAcc.tile([1, nq], F32)
                nc.tensor.matmul(
                    s_ps[:], lhsT=onesc_f32[:], rhs=s_sum[:],
                    start=True, stop=True,
                )
                s_sb = cpool.tile([1, nq], F32)
                nc.vector.tensor_copy(s_sb[:], s_ps[:])
                rinv_sb = cpool.tile([1, nq], F32)
                nc.vector.reciprocal(rinv_sb[:], s_sb[:])
                nc.vector.tensor_scalar_mul(rinv_sb[:], rinv_sb[:], SZ)
                rb_ps = psA.tile([128, nq], F32, tag="att")
                nc.tensor.matmul(
                    rb_ps[:], lhsT=ones1_f32[:], rhs=rinv_sb[:],
                    start=True, stop=True,
                )
                rb_sb = cpool.tile([128, nq], F32)
                nc.vector.tensor_copy(rb_sb[:], rb_ps[:])
                zq_sb = cpool.tile([128, DS, nq], FP8)
                for ds in range(DS):
                    nc.vector.tensor_tensor(
                        zq_sb[:, ds], zT_ps[:, ds], rb_sb[:], ALU.mult
                    )
                nc.sync.dma_start(zq_d[:], zq_sb[:])

                # ---- classifier + CE (fp8, weight-stationary over chunks) ----
                sep_sb = cpool.tile([128, qt, waves], F32)
                if no_ce:
                    nc.vector.memset(sep_sb[:], 1.0)
                else:
                    for q in range(qt):
                        for wv in range(waves):
                            g2_ps = psA.tile([128, 2, 512], F32, tag="att")
                            for dp in range(2):
                                for k in range(2):
                                    c0 = (wv * 2 + k) * CW
                                    nc.tensor.matmul(
                                        g2_ps[:, k, :CW],
                                        lhsT=zq_sb[:, 2 * dp:2 * dp + 2,
                                                   q * 128:(q + 1) * 128],
                                        rhs=fcw_sb[:, 2 * dp:2 * dp + 2,
                                                   c0:c0 + CW],
                                        start=(dp == 0),
                                        stop=(dp == 1 and not with_bias),
                                        perf_mode=DR,
                                    )
                            if with_bias:
                                for k in range(2):
                                    c0 = (wv * 2 + k) * CW
                                    nc.tensor.matmul(
                                        g2_ps[:, k, :CW],
                                        lhsT=ones1_bf[:],
                                        rhs=fcb_sb[:, c0:c0 + CW],
                                        start=False,
                                        stop=True,
                                    )
                            e_sb = wpool.tile([128, 2, 512], BF16, tag="e")
                            nc.scalar.activation(
                                e_sb[:, :, :CW],
                                g2_ps[:, :, :CW],
                                AF.Exp,
                                scale=1.0 / (SZ * SF),
                                accum_out=sep_sb[:, q, wv:wv + 1],
                            )

                # lse and the target-logit dot are finished on the host from
                # sep and zq (tiny transfers; avoids a serial on-device tail)
                nc.sync.dma_start(sep_d[:], sep_sb[:])

    nc.compile()
    return nc


def _q8(x, scale=1.0):
    e4 = ml_dtypes.float8_e4m3
    return np.clip(np.asarray(x, np.float32) * scale, -240.0, 240.0).astype(e4)


def prep_inputs(q, db_vecs, db_labels, fc_w, fc_b, nst=NST, nch=NCH, nq=NQ,
                n_cores=N_CORES, with_bias=False):
    """Host-side sharding / layout prep.  Returns per-core input maps."""
    qt = nq // 128
    n_classes = nch * CW

    # shared (core-independent) layouts
    dbT_h = _q8(np.ascontiguousarray(
        db_vecs.reshape(nst, 4, 128, DS, 128).transpose(0, 4, 1, 3, 2)
    ))                                                   # [st, p(d_in), j, ds, n']
    db_h = _q8(np.ascontiguousarray(
        db_vecs.reshape(nst, 4, 128, D).transpose(0, 2, 1, 3)
    ))                                                   # [st, n', j, d]
    fcw_h = _q8(np.ascontiguousarray(
        fc_w.T.reshape(DS, 128, n_classes).transpose(1, 0, 2)
    ), scale=SF)                                         # [p(d_in), ds, c]

    in_maps = []
    for core in range(n_cores):
        q_c = q[core * nq:(core + 1) * nq]               # [nq, D]
        qT_h = _q8(np.ascontiguousarray(
            q_c.T.reshape(DS, 128, nq).transpose(1, 0, 2)
        ))                                               # [p(d_in), ds, q]
        m = {"qT": qT_h, "dbT": dbT_h, "db": db_h, "fcw": fcw_h}
        if with_bias:
            m["fcb"] = (fc_b.reshape(1, n_classes) * (SZ * SF)).astype(
                ml_dtypes.bfloat16)
        in_maps.append(m)
    return in_maps


def finish_host(res_core, labels_core, fc_w, fc_b, nq=NQ):
    """Combine a core's sep / zq outputs into per-query nll (f32 host math)."""
    qt = nq // 128
    sep = np.asarray(res_core["sep"], np.float32)        # [128, qt, waves]
    lse = np.log(sep.sum(axis=2)).T.reshape(-1)          # [nq] (q = qt*128+p)
    zq = np.asarray(res_core["zq"], np.float32)          # [128(d'), DS, nq]
    zvals = zq.transpose(2, 1, 0).reshape(nq, D)         # [q, d]
    wt = fc_w[labels_core]                               # [nq, D]
    tl = (zvals * wt).sum(axis=1) / SZ
    if np.any(fc_b):
        tl = tl + fc_b[labels_core]
    return lse - tl


def kernel(q, db_vecs, db_labels, fc_w, fc_b, _return_results=False, **run_kwargs):
    q = np.asarray(q, np.float32)
    db_vecs = np.asarray(db_vecs, np.float32)
    fc_w = np.asarray(fc_w, np.float32)
    fc_b = np.asarray(fc_b, np.float32)

    with_bias = bool(np.any(fc_b))
    key = ("nc", with_bias)
    if key not in _CACHE:
        _CACHE[key] = build_nc(with_bias=with_bias)
    nc = _CACHE[key]

    in_maps = prep_inputs(q, db_vecs, db_labels, fc_w, fc_b,
                          with_bias=with_bias)
    res = run_bass_kernel_spmd(nc, in_maps, core_ids=list(range(N_CORES)),
                               **run_kwargs)
    labels = np.asarray(db_labels).reshape(-1).astype(np.int64)
    nlls = [
        finish_host(r, labels[c * NQ:(c + 1) * NQ], fc_w, fc_b)
        for c, r in enumerate(res.results)
    ]
    out = np.float32(np.mean(np.concatenate(nlls)))
    if _return_results:
        return out, res
    return out


# revision 21
# speedup vs baseline: 2.5241x; 2.5241x over previous
"""Trainium2 Bass kernel for nn_CACLayer (retrieval + softmax readout + CE).

Computation (see reference):
  att = (q @ db.T) / sqrt(D); w = softmax(att, -1); z = w @ db
  logits = z @ fc_w.T + fc_b; nll = -log_softmax(logits)[targets]; out = mean(nll)

Strategy: data-parallel over batch B=2048 across 8 cores (256 queries each).
All heavy matmuls run in fp8e4m3 with DoubleRow perf mode (k=256 contraction,
~1.7x the bf16 column rate).  Softmax is computed un-normalized with the exp
biased by -ln(32) so the fp8 weights stay inside e4m3 range (softmax is
invariant to that scaling); the softmax sums accumulate on the DVE and Pool
engines so the PE only runs the att / z / classifier matmuls.  z is
normalized and rescaled by SZ=128 into fp8; fc_w is pre-scaled by SF=32 on
the host; the classifier exp folds the 1/(SZ*SF) back in via its scale.
"""

import os
import sys

for _p in ("/opt/trn_rl_repo", "/root/.axon_site/_ro/trn_rl_repo"):
    if os.path.isdir(_p) and _p not in sys.path:
        sys.path.insert(0, _p)

import math
import numpy as np
import ml_dtypes

from concourse import bacc, mybir, tile
from concourse.bass_utils import run_bass_kernel_spmd

BF16 = mybir.dt.bfloat16
F32 = mybir.dt.float32
FP8 = mybir.dt.float8e4
AF = mybir.ActivationFunctionType
ALU = mybir.AluOpType
DR = mybir.MatmulPerfMode.DoubleRow

D = 512          # embed dim
N_DB = 32768     # database rows
B = 2048         # batch
C = 10000        # classes
N_CORES = 8
NQ = B // N_CORES          # queries per core (256)
QT = NQ // 128             # q tiles per core (2)
DS = D // 128              # d slices (4)
NST = N_DB // 512          # supertiles of 4 n-tiles (64)
TAU = float(D) ** -0.5
WBIAS = -math.log(32.0)    # exp output scale: keeps w = exp(att)/32 < 240
SZ = 128.0                 # z quantization scale
SF = 32.0                  # fc_w quantization scale
CW = 500                   # classifier chunk width (20 chunks x 500 = C)
NCH = C // CW              # 20 chunks
WAVES = NCH // 2           # CE waves of 2 chunks

_CACHE = {}


def build_nc(nst=NST, nch=NCH, nq=NQ, repeat=1, stream_bufs=4, wexp_bufs=3,
             no_ce=False, no_s=False, no_dma=False, with_bias=False):
    """Build the Bass module.  Parameterized so a scaled-down version can be
    simulated; hardware uses the defaults."""
    qt = nq // 128
    n_classes = nch * CW
    waves = nch // 2

    nc = bacc.Bacc("TRN2", target_bir_lowering=False, debug=False)

    qT_d = nc.dram_tensor("qT", [128, DS, nq], FP8, kind="ExternalInput")
    dbT_d = nc.dram_tensor("dbT", [nst, 128, 4, DS, 128], FP8, kind="ExternalInput")
    db_d = nc.dram_tensor("db", [nst, 128, 4, D], FP8, kind="ExternalInput")
    fcw_d = nc.dram_tensor("fcw", [128, DS, n_classes], FP8, kind="ExternalInput")
    sep_d = nc.dram_tensor("sep", [128, qt, waves], F32, kind="ExternalOutput")
    zq_d = nc.dram_tensor("zq", [128, DS, nq], FP8, kind="ExternalOutput")
    if with_bias:
        fcb_d = nc.dram_tensor("fcb", [1, n_classes], BF16, kind="ExternalInput")

    with tile.TileContext(nc) as tc:
        with (
            tc.tile_pool(name="const", bufs=1) as cpool,
            tc.tile_pool(name="stream", bufs=stream_bufs) as spool,
            tc.tile_pool(name="wexp", bufs=wexp_bufs) as wpool,
            tc.tile_pool(name="psA", bufs=2, space="PSUM") as psA,
            tc.tile_pool(name="psAcc", bufs=1, space="PSUM") as psAcc,
        ):
            # ---- resident tensors ----
            qT_sb = cpool.tile([128, DS, nq], FP8)
            nc.sync.dma_start(qT_sb[:], qT_d[:])
            fcw_sb = cpool.tile([128, DS, n_classes], FP8)
            for ds in range(DS):
                nc.sync.dma_start(fcw_sb[:, ds], fcw_d[:, ds])
            if with_bias:
                fcb_sb = cpool.tile([1, n_classes], BF16)
                nc.sync.dma_start(fcb_sb[:], fcb_d[:])
                ones1_bf = cpool.tile([1, 128], BF16)
                nc.vector.memset(ones1_bf[:], 1.0)

            onesc_f32 = cpool.tile([128, 1], F32)
            nc.vector.memset(onesc_f32[:], 1.0)
            ones1_f32 = cpool.tile([1, 128], F32)
            nc.vector.memset(ones1_f32[:], 1.0)
            wbias_sb = cpool.tile([128, 1], F32)
            nc.vector.memset(wbias_sb[:], WBIAS)

            # ---- phase A: att -> exp -> z accumulation over db ----
            # zT_ps [128(d_in), DS, nq] : 2 psum banks
            zT_ps = psAcc.tile([128, DS, nq], F32)

            import contextlib
            rep_cm = tc.For_i(0, repeat, 1) if repeat > 1 else contextlib.nullcontext()
            with rep_cm:
                # 2-stage software pipeline over supertiles: while the ACT
                # engine computes exp(st), the PE runs att matmuls of st+1,
                # so the PE never stalls on the exp dependency.
                db_tiles = {}

                def _load(st):
                    if no_dma and st > 0:
                        db_tiles[st] = db_tiles[0]
                        return
                    dbT_sb = spool.tile([128, 4, DS, 128], FP8, tag="dbT")
                    db_sb = spool.tile([128, 4, D], FP8, tag="db")
                    nc.sync.dma_start(dbT_sb[:], dbT_d[st])
                    nc.sync.dma_start(db_sb[:], db_d[st])
                    db_tiles[st] = (dbT_sb, db_sb)

                att_tiles = {}

                def _att(st):
                    dbT_sb = db_tiles[st][0]
                    att_ps = psA.tile([128, 4, nq], F32, tag="att")
                    # att_ps spans 2 psum banks (j 0,1 / j 2,3): one
                    # accumulation group per bank
                    for j in range(4):
                        for dp in range(2):
                            nc.tensor.matmul(
                                att_ps[:, j, :],
                                lhsT=dbT_sb[:, j, 2 * dp:2 * dp + 2, :],
                                rhs=qT_sb[:, 2 * dp:2 * dp + 2, :],
                                start=(dp == 0 and j % 2 == 0),
                                stop=(dp == 1 and j % 2 == 1),
                                perf_mode=DR,
                            )
                    att_tiles[st] = att_ps

                # softmax-sum accumulators (DVE + Pool), reset via copy at st=0
                s_accv = cpool.tile([128, nq], F32)
                s_accp = cpool.tile([128, nq], F32)

                _load(0)
                _load(1)
                _att(0)
                for st in range(nst):
                    if st + 2 < nst:
                        _load(st + 2)
                    w_sb = wpool.tile([128, 4, nq], FP8, tag="w")
                    nc.scalar.activation(
                        w_sb[:], att_tiles.pop(st)[:], AF.Exp, scale=TAU,
                        bias=wbias_sb[:],
                    )
                    if st + 1 < nst:
                        _att(st + 1)
                    db_sb = db_tiles[st][1]
                    if not no_dma:
                        del db_tiles[st]
                    for jp in range(2):
                        for ds in range(DS):
                            nc.tensor.matmul(
                                zT_ps[:, ds, :],
                                lhsT=db_sb[:, 2 * jp:2 * jp + 2,
                                           ds * 128:(ds + 1) * 128],
                                rhs=w_sb[:, 2 * jp:2 * jp + 2, :],
                                start=(st == 0 and jp == 0 and ds in (0, 2)),
                                stop=(st == nst - 1 and jp == 1
                                      and ds in (1, 3)),
                                perf_mode=DR,
                            )
                    if not no_s:
                        if st == 0:
                            nc.vector.tensor_copy(s_accv[:], w_sb[:, 0, :])
                            nc.vector.tensor_tensor(
                                s_accv[:], s_accv[:], w_sb[:, 1, :], ALU.add)
                            nc.gpsimd.tensor_copy(s_accp[:], w_sb[:, 2, :])
                            nc.gpsimd.tensor_tensor(
                                s_accp[:], s_accp[:], w_sb[:, 3, :], ALU.add)
                        else:
                            for j, eng, acc in ((0, nc.vector, s_accv),
                                                (1, nc.vector, s_accv),
                                                (2, nc.gpsimd, s_accp),
                                                (3, nc.gpsimd, s_accp)):
                                eng.tensor_tensor(
                                    acc[:], acc[:], w_sb[:, j, :], ALU.add)

                # ---- softmax normalization of z (scaled by SZ into fp8) ----
                s_sum = cpool.tile([128, nq], F32)
                if no_s:
                    nc.vector.memset(s_sum[:], 1.0)
                else:
                    nc.vector.tensor_tensor(
                        s_sum[:], s_accv[:], s_accp[:], ALU.add)
                s_ps = psAcc.tile([1, nq], F32)
                nc.tensor.matmul(
                    s_ps[:], lhsT=onesc_f32[:], rhs=s_sum[:],
                    start=True, stop=True,
                )
                s_sb = cpool.tile([1, nq], F32)
                nc.vector.tensor_copy(s_sb[:], s_ps[:])
                rinv_sb = cpool.tile([1, nq], F32)
                nc.vector.reciprocal(rinv_sb[:], s_sb[:])
                nc.vector.tensor_scalar_mul(rinv_sb[:], rinv_sb[:], SZ)
                rb_ps = psA.tile([128, nq], F32, tag="att")
                nc.tensor.matmul(
                    rb_ps[:], lhsT=ones1_f32[:], rhs=rinv_sb[:],
                    start=True, stop=True,
                )
                rb_sb = cpool.tile([128, nq], F32)
                nc.vector.tensor_copy(rb_sb[:], rb_ps[:])
                zq_sb = cpool.tile([128, DS, nq], FP8)
                for ds in range(DS):
                    nc.vector.tensor_tensor(
                        zq_sb[:, ds], zT_ps[:, ds], rb_sb[:], ALU.mult
                    )
                nc.sync.dma_start(zq_d[:], zq_sb[:])

                # ---- classifier + CE (fp8, weight-stationary over chunks) ----
                sep_sb = cpool.tile([128, qt, waves], F32)
                if no_ce:
                    nc.vector.memset(sep_sb[:], 1.0)
                else:
                    for q in range(qt):
                        for wv in range(waves):
                            g2_ps = psA.tile([128, 2, 512], F32, tag="att")
                            for dp in range(2):
                                for k in range(2):
                                    c0 = (wv * 2 + k) * CW
                                    nc.tensor.matmul(
                                        g2_ps[:, k, :CW],
                                        lhsT=zq_sb[:, 2 * dp:2 * dp + 2,
                                                   q * 128:(q + 1) * 128],
                                        rhs=fcw_sb[:, 2 * dp:2 * dp + 2,
                                                   c0:c0 + CW],
                                        start=(dp == 0),
                                        stop=(dp == 1 and not with_bias),
                                        perf_mode=DR,
                                    )
                            if with_bias:
                                for k in range(2):
                                    c0 = (wv * 2 + k) * CW
                                    nc.tensor.matmul(
                                        g2_ps[:, k, :CW],
                                        lhsT=ones1_bf[:],
                                        rhs=fcb_sb[:, c0:c0 + CW],
                                        start=False,
                                        stop=True,
                                    )
                            e_sb = wpool.tile([128, 2, 512], BF16, tag="e")
                            nc.scalar.activation(
                                e_sb[:, :, :CW],
                                g2_ps[:, :, :CW],
                                AF.Exp,
                                scale=1.0 / (SZ * SF),
                                accum_out=sep_sb[:, q, wv:wv + 1],
                            )

                # lse and the target-logit dot are finished on the host from
                # sep and zq (tiny transfers; avoids a serial on-device tail)
                nc.sync.dma_start(sep_d[:], sep_sb[:])

    nc.compile()
    return nc


def _q8(x, scale=1.0):
    e4 = ml_dtypes.float8_e4m3
    return np.clip(np.asarray(x, np.float32) * scale, -240.0, 240.0).astype(e4)


def prep_inputs(q, db_vecs, db_labels, fc_w, fc_b, nst=NST, nch=NCH, nq=NQ,
                n_cores=N_CORES, with_bias=False):
    """Host-side sharding / layout prep.  Returns per-core input maps."""
    qt = nq // 128
    n_classes = nch * CW

    # shared (core-independent) layouts
    dbT_h = _q8(np.ascontiguousarray(
        db_vecs.reshape(nst, 4, 128, DS, 128).transpose(0, 4, 1, 3, 2)
    ))                                                   # [st, p(d_in), j, ds, n']
    db_h = _q8(np.ascontiguousarray(
        db_vecs.reshape(nst, 4, 128, D).transpose(0, 2, 1, 3)
    ))                                                   # [st, n', j, d]
    fcw_h = _q8(np.ascontiguousarray(
        fc_w.T.reshape(DS, 128, n_classes).transpose(1, 0, 2)
    ), scale=SF)                                         # [p(d_in), ds, c]

    in_maps = []
    for core in range(n_cores):
        q_c = q[core * nq:(core + 1) * nq]               # [nq, D]
        qT_h = _q8(np.ascontiguousarray(
            q_c.T.reshape(DS, 128, nq).transpose(1, 0, 2)
        ))                                               # [p(d_in), ds, q]
        m = {"qT": qT_h, "dbT": dbT_h, "db": db_h, "fcw": fcw_h}
        if with_bias:
            m["fcb"] = (fc_b.reshape(1, n_classes) * (SZ * SF)).astype(
                ml_dtypes.bfloat16)
        in_maps.append(m)
    return in_maps


def finish_host(res_core, labels_core, fc_w, fc_b, nq=NQ):
    """Combine a core's sep / zq outputs into per-query nll (f32 host math)."""
    qt = nq // 128
    sep = np.asarray(res_core["sep"], np.float32)        # [128, qt, waves]
    lse = np.log(sep.sum(axis=2)).T.reshape(-1)          # [nq] (q = qt*128+p)
    zq = np.asarray(res_core["zq"], np.float32)          # [128(d'), DS, nq]
    zvals = zq.transpose(2, 1, 0).reshape(nq, D)         # [q, d]
    wt = fc_w[labels_core]                               # [nq, D]
    tl = (zvals * wt).sum(axis=1) / SZ
    if np.any(fc_b):
        tl = tl + fc_b[labels_core]
    return lse - tl


def kernel(q, db_vecs, db_labels, fc_w, fc_b, _return_results=False, **run_kwargs):
    q = np.asarray(q, np.float32)
    db_vecs = np.asarray(db_vecs, np.float32)
    fc_w = np.asarray(fc_w, np.float32)
    fc_b = np.asarray(fc_b, np.float32)

    with_bias = bool(np.any(fc_b))
    key = ("nc", with_bias)
    if key not in _CACHE:
        _CACHE[key] = build_nc(with_bias=with_bias)
    nc = _CACHE[key]

    in_maps = prep_inputs(q, db_vecs, db_labels, fc_w, fc_b,
                          with_bias=with_bias)
    res = run_bass_kernel_spmd(nc, in_maps, core_ids=list(range(N_CORES)),
                               **run_kwargs)
    labels = np.asarray(db_labels).reshape(-1).astype(np.int64)
    nlls = [
        finish_host(r, labels[c * NQ:(c + 1) * NQ], fc_w, fc_b)
        for c, r in enumerate(res.results)
    ]
    out = np.float32(np.mean(np.concatenate(nlls)))
    if _return_results:
        return out, res
    return out
